# revision 1
# baseline (speedup 1.0000x reference)
"""Trainium2 Bass kernel for nn_CategoricalAwareTabularEncoder.

Data-parallel over batch: 8 cores x 32 batches each. Per core:
  - indirect-DMA gather of embedding rows + category frequencies
  - PE transposes into emb_aug [66, 2048] (rows 0-63 emb.T, 64 ones, 65 freqs)
  - pairwise-combo via delta decomposition:
      Hsum = sum_{d=1..63} sum_i relu(A'_i + B_{i+d}),  A' = emb@W1top + b1,
      B = emb@W1bot  (two DVE ops per delta on [128, 16, 64-d] tiles)
  - freq MLP, cluster softmax; biases folded into augmented matmuls.
All small parameters arrive in one packed DRAM tensor (single DMA) to keep
per-instruction sync-wait counts low.
"""
import sys

if "/opt/trn_rl_repo" not in sys.path:
    sys.path.insert(0, "/opt/trn_rl_repo")

import numpy as np

import concourse.bass as bass
import concourse.bacc as bacc
import concourse.mybir as mybir
import concourse.tile as tile
from concourse.masks import make_identity

FP = mybir.dt.float32
I32 = mybir.dt.int32
Alu = mybir.AluOpType
Act = mybir.ActivationFunctionType
Ax = mybir.AxisListType

NCORES = 8
B, S, V, D, F = 256, 64, 100000, 64, 10
B_LOC = B // NCORES            # 32 batches per core
N = B_LOC * S                  # 2048 tokens per core
NCH = N // 128                 # 16 chunks of 128 tokens
HB = B_LOC // 2                # 16 batches per partition-half
PAIRS = S * (S - 1) // 2       # 2016
INV = 1.0 / (PAIRS + 2)

# wpack column layout (all f32, [128, WCOLS]):
C_W1T = 0       # [64, 64] comb_w1[:64]
C_W1B = 64      # [64, 64] comb_w1[64:]
C_FW1T = 128    # [64, 64] freq_w1[:64]
C_W2F = 192     # [64, 10] freq_w2
C_W2C = 202     # [64, 10] comb_w2
C_CENT = 212    # [64, 10] cluster_centers.T
C_CB1 = 222     # row 0: [1, 64] comb_b1
C_FB1 = 286     # row 0: [1, 64] freq_b1
C_B2 = 350      # row 0: [1, 10] comb_b2
C_FB2 = 360     # row 0: [1, 10] freq_b2
C_FW1L = 222    # row 64: [1, 64] freq_w1[64]
C_TS = 286      # row 64: [1, 1] total_samples
WCOLS = 384


def _build():
    nc = bass.Bass()
    ids = nc.dram_tensor("ids", [N], I32, kind="ExternalInput")
    emb_table = nc.dram_tensor("emb_table", [V, D], FP, kind="ExternalInput")
    cat_freq = nc.dram_tensor("cat_freq", [V, 1], FP, kind="ExternalInput")
    wpack_d = nc.dram_tensor("wpack", [128, WCOLS], FP, kind="ExternalInput")
    out = nc.dram_tensor("out", [N, F], FP, kind="ExternalOutput")

    with tile.TileContext(nc) as tc:
        with (
            tc.tile_pool(name="cpool", bufs=1) as cpool,
            tc.tile_pool(name="wpool", bufs=3) as wpool,
            tc.tile_pool(name="ppool", bufs=2, space="PSUM") as ppool,
        ):
            ident = cpool.tile([128, 128], FP)
            make_identity(nc, ident[:])

            wp = cpool.tile([128, WCOLS], FP)
            nc.sync.dma_start(out=wp[:], in_=wpack_d[:])
            ids_sb = cpool.tile([128, NCH], I32)
            nc.sync.dma_start(
                out=ids_sb[:], in_=ids[:].rearrange("(c p) -> p c", p=128)
            )
            w1t = wp[0:D, C_W1T : C_W1T + D]
            w1b = wp[0:D, C_W1B : C_W1B + D]
            w2f = wp[0:D, C_W2F : C_W2F + F]
            w2c = wp[0:D, C_W2C : C_W2C + F]
            centT = wp[0:D, C_CENT : C_CENT + F]
            cb1 = wp[0:1, C_CB1 : C_CB1 + D]
            fb1 = wp[0:1, C_FB1 : C_FB1 + D]

            # ones row for rank-1 bias matmuls
            ones1N = cpool.tile([1, N], FP)
            nc.vector.memset(ones1N[:], 1.0)

            # freq-MLP lhsT [65, 64]: rows 0-63 fW1top, row 64 = fW1last/total
            lhsFr = cpool.tile([D + 1, D], FP)
            nc.scalar.activation(out=lhsFr[0:D, :], in_=wp[0:D, C_FW1T : C_FW1T + D], func=Act.Copy)
            rec65 = cpool.tile([D + 1, 1], FP)
            nc.vector.reciprocal(
                out=rec65[D : D + 1, :], in_=wp[D : D + 1, C_TS : C_TS + 1]
            )
            nc.vector.tensor_scalar_mul(
                out=lhsFr[D : D + 1, :], in0=wp[D : D + 1, C_FW1L : C_FW1L + D],
                scalar1=rec65[D : D + 1, :],
            )

            # combo bias column: b2col[f] = freq_b2[f] + PAIRS*comb_b2[f]
            b2comb = wpool.tile([1, F], FP, tag="b2comb")
            nc.vector.scalar_tensor_tensor(
                out=b2comb[:], in0=wp[0:1, C_B2 : C_B2 + F],
                scalar=float(PAIRS), in1=wp[0:1, C_FB2 : C_FB2 + F],
                op0=Alu.mult, op1=Alu.add,
            )
            b2p = ppool.tile([F, 1], FP, space="PSUM", tag="tiny", bufs=1)
            nc.tensor.transpose(out=b2p[:], in_=b2comb[:], identity=ident[0:1, 0:1])
            b2col = cpool.tile([F, 1], FP)
            nc.scalar.activation(out=b2col[:], in_=b2p[:], func=Act.Copy)

            # cluster weights: cent_sb = -2*centers.T; csq_sb = |c|^2 row
            cent_sb = cpool.tile([D, F], FP)
            nc.scalar.activation(out=cent_sb[:], in_=centT, func=Act.Copy, scale=-2.0)
            sqT = wpool.tile([D, F], FP, tag="sqT")
            nc.vector.tensor_mul(out=sqT[:], in0=centT, in1=centT)
            ones64 = cpool.tile([D, 1], FP)
            nc.vector.memset(ones64[:], 1.0)
            csqp = ppool.tile([1, F], FP, space="PSUM", tag="tiny", bufs=1)
            nc.tensor.matmul(out=csqp[:], lhsT=ones64[:], rhs=sqT[:], start=True, stop=True)
            csq_sb = cpool.tile([1, F], FP)
            nc.scalar.activation(out=csq_sb[:], in_=csqp[:], func=Act.Copy)

            # ---- gather + transpose into emb_aug [65, N] -------------------
            # rows 0-63: emb.T, row 64: gathered cat_freq (raw)
            emb_aug = cpool.tile([D + 1, N], FP)
            esq = cpool.tile([128, NCH], FP)
            for c in range(NCH):
                rows = wpool.tile([128, D + 1], FP, tag="rows")
                nc.gpsimd.indirect_dma_start(
                    out=rows[:, 0:D], out_offset=None, in_=emb_table[:],
                    in_offset=bass.IndirectOffsetOnAxis(ap=ids_sb[:, c : c + 1], axis=0),
                )
                nc.gpsimd.indirect_dma_start(
                    out=rows[:, D : D + 1], out_offset=None, in_=cat_freq[:],
                    in_offset=bass.IndirectOffsetOnAxis(ap=ids_sb[:, c : c + 1], axis=0),
                )
                sqr = wpool.tile([128, D], FP, tag="sqr")
                nc.vector.tensor_mul(out=sqr[:], in0=rows[:, 0:D], in1=rows[:, 0:D])
                nc.vector.tensor_reduce(
                    out=esq[:, c : c + 1], in_=sqr[:], axis=Ax.X, op=Alu.add
                )
                tp = ppool.tile([D + 1, 128], FP, space="PSUM", tag="tp")
                nc.tensor.transpose(out=tp[:], in_=rows[:], identity=ident[:])
                cols = slice(128 * c, 128 * (c + 1))
                nc.scalar.activation(out=emb_aug[0:D, cols], in_=tp[0:D, :], func=Act.Copy)
                nc.scalar.activation(
                    out=emb_aug[D : D + 1, cols], in_=tp[D : D + 1, :], func=Act.Copy
                )

            # ---- A' and B activations, two batch-halves stacked ------------
            A2 = cpool.tile([128, HB * S], FP)
            B2 = cpool.tile([128, HB * S], FP)
            for jb in range(2):
                dst = slice(512 * jb, 512 * (jb + 1))
                lo = slice(512 * jb, 512 * (jb + 1))
                hi = slice(1024 + 512 * jb, 1024 + 512 * (jb + 1))
                pa = ppool.tile([128, 512], FP, space="PSUM", tag="mm512")
                nc.tensor.matmul(out=pa[0:D, :], lhsT=w1t, rhs=emb_aug[0:D, lo], start=True, stop=False)
                nc.tensor.matmul(out=pa[0:D, :], lhsT=cb1, rhs=ones1N[0:1, lo], start=False, stop=True)
                nc.tensor.matmul(out=pa[D:128, :], lhsT=w1t, rhs=emb_aug[0:D, hi], start=True, stop=False)
                nc.tensor.matmul(out=pa[D:128, :], lhsT=cb1, rhs=ones1N[0:1, hi], start=False, stop=True)
                nc.scalar.activation(out=A2[:, dst], in_=pa[:], func=Act.Copy)
                pb = ppool.tile([128, 512], FP, space="PSUM", tag="mm512")
                nc.tensor.matmul(out=pb[0:D, :], lhsT=w1b, rhs=emb_aug[0:D, lo], start=True, stop=True)
                nc.tensor.matmul(out=pb[D:128, :], lhsT=w1b, rhs=emb_aug[0:D, hi], start=True, stop=True)
                nc.scalar.activation(out=B2[:, dst], in_=pb[:], func=Act.Copy)

            # ---- pair loop: acc[i] += relu(A'_i + B_{i+d}) -----------------
            acc = cpool.tile([128, HB * S], FP)
            nc.vector.memset(acc[:], 0.0)
            A3 = A2[:].rearrange("p (b i) -> p b i", i=S)
            B3 = B2[:].rearrange("p (b i) -> p b i", i=S)
            C3 = acc[:].rearrange("p (b i) -> p b i", i=S)
            for d in range(1, S):
                L = S - d
                tmp = wpool.tile([128, HB * S], FP, tag="ptmp")
                t3 = tmp[:].rearrange("p (b i) -> p b i", i=S)
                nc.vector.tensor_tensor(
                    out=t3[:, :, 0:L], in0=A3[:, :, 0:L], in1=B3[:, :, d:S], op=Alu.add
                )
                nc.vector.scalar_tensor_tensor(
                    out=C3[:, :, 0:L], in0=t3[:, :, 0:L], scalar=0.0,
                    in1=C3[:, :, 0:L], op0=Alu.max, op1=Alu.add,
                )
            hsum = cpool.tile([128, HB], FP)
            nc.vector.tensor_reduce(out=hsum[:], in_=C3[:], axis=Ax.X, op=Alu.add)

            # ---- combo_sum: [10, 32] (+bias col) -> transpose -> [32, 10] --
            hsum_hi = cpool.tile([D, HB], FP)
            nc.sync.dma_start(out=hsum_hi[:], in_=hsum[D:128, :])
            pc = ppool.tile([F, B_LOC], FP, space="PSUM", tag="tiny", bufs=1)
            nc.tensor.matmul(out=pc[:, 0:HB], lhsT=w2c, rhs=hsum[0:D, :], start=True, stop=True)
            nc.tensor.matmul(out=pc[:, HB:B_LOC], lhsT=w2c, rhs=hsum_hi[:], start=True, stop=True)
            cs1 = wpool.tile([F, B_LOC], FP, tag="cs1")
            nc.vector.tensor_scalar_add(out=cs1[:], in0=pc[:], scalar1=b2col[:])
            pt = ppool.tile([B_LOC, F], FP, space="PSUM", tag="tiny", bufs=1)
            nc.tensor.transpose(out=pt[:], in_=cs1[:], identity=ident[0:F, 0:F])
            combo_sb = cpool.tile([B_LOC, F], FP)
            nc.scalar.activation(out=combo_sb[:], in_=pt[:], func=Act.Copy, scale=INV)

            # block-diagonal selector: sel[b, n] = 1 iff n // 64 == b
            sel = cpool.tile([B_LOC, N], FP)
            nc.gpsimd.memset(sel[:], 1.0)
            nc.gpsimd.affine_select(
                out=sel[:], in_=sel[:], pattern=[[1, N]], compare_op=Alu.is_ge,
                fill=0.0, base=0, channel_multiplier=-S,
            )
            nc.gpsimd.affine_select(
                out=sel[:], in_=sel[:], pattern=[[-1, N]], compare_op=Alu.is_ge,
                fill=0.0, base=S - 1, channel_multiplier=S,
            )

            # ---- freq hidden: fh [64, N] = INV*relu(...) -------------------
            fh = cpool.tile([D, N], FP)
            for q in range(4):
                sq_ = slice(512 * q, 512 * (q + 1))
                pf = ppool.tile([D, 512], FP, space="PSUM", tag="mm512")
                nc.tensor.matmul(
                    out=pf[:], lhsT=lhsFr[:], rhs=emb_aug[0 : D + 1, sq_],
                    start=True, stop=False,
                )
                nc.tensor.matmul(
                    out=pf[:], lhsT=fb1, rhs=ones1N[0:1, sq_], start=False, stop=True
                )
                # relu(x*INV) == INV*relu(x) since INV > 0
                nc.scalar.activation(out=fh[:, sq_], in_=pf[:], func=Act.Relu, scale=INV)

            # ---- per-chunk: freq_feat+combo (PE) + cluster softmax ---------
            osb = cpool.tile([128, NCH * F], FP)
            for c in range(NCH):
                cols = slice(128 * c, 128 * (c + 1))
                pff = ppool.tile([128, F], FP, space="PSUM", tag="pff")
                nc.tensor.matmul(out=pff[:], lhsT=fh[:, cols], rhs=w2f, start=True, stop=False)
                nc.tensor.matmul(out=pff[:], lhsT=sel[:, cols], rhs=combo_sb[:], start=False, stop=True)
                psc = ppool.tile([128, F], FP, space="PSUM", tag="psc", bufs=1)
                nc.tensor.matmul(
                    out=psc[:], lhsT=emb_aug[0:D, cols], rhs=cent_sb[:],
                    start=True, stop=False,
                )
                nc.tensor.matmul(
                    out=psc[:], lhsT=ones1N[0:1, cols], rhs=csq_sb[:],
                    start=False, stop=True,
                )
                dist = wpool.tile([128, F], FP, tag="dist")
                nc.vector.tensor_scalar_add(
                    out=dist[:], in0=psc[:], scalar1=esq[:, c : c + 1]
                )
                nc.vector.tensor_scalar_max(out=dist[:], in0=dist[:], scalar1=0.0)
                nc.scalar.activation(out=dist[:], in_=dist[:], func=Act.Sqrt)
                mn = wpool.tile([128, 1], FP, tag="mn")
                nc.vector.tensor_reduce(out=mn[:], in_=dist[:], axis=Ax.X, op=Alu.min)
                ex = wpool.tile([128, F], FP, tag="ex")
                nc.scalar.activation(
                    out=ex[:], in_=dist[:], func=Act.Exp, bias=mn[:], scale=-1.0
                )
                sm = wpool.tile([128, 1], FP, tag="sm")
                nc.vector.tensor_reduce(out=sm[:], in_=ex[:], axis=Ax.X, op=Alu.add)
                nc.vector.tensor_scalar_mul(
                    out=sm[:], in0=sm[:], scalar1=float(PAIRS + 2)
                )
                rc = wpool.tile([128, 1], FP, tag="rc")
                nc.vector.reciprocal(out=rc[:], in_=sm[:])
                nc.vector.scalar_tensor_tensor(
                    out=osb[:, F * c : F * (c + 1)], in0=ex[:], scalar=rc[:],
                    in1=pff[:], op0=Alu.mult, op1=Alu.add,
                )
            nc.sync.dma_start(
                out=out[:].rearrange("(c p) f -> p c f", p=128),
                in_=osb[:].rearrange("p (c f) -> p c f", f=F),
            )
    # Split multi-sem waits to satisfy the 1-wait-per-instruction HW limit
    # (normally done by Bacc.compile; plain Bass skips it).
    import bass_rust as _br
    _br.move_matmul_waits_to_ldweights(nc.m)
    _br.generate_event_semaphores(nc)
    return nc


_NC = None
_last_in_maps = None


def _pack_weights(inputs):
    f32 = lambda k: np.asarray(inputs[k], dtype=np.float32)
    wp = np.zeros((128, WCOLS), np.float32)
    wp[0:D, C_W1T : C_W1T + D] = f32("comb_w1")[0:D]
    wp[0:D, C_W1B : C_W1B + D] = f32("comb_w1")[D : 2 * D]
    wp[0:D, C_FW1T : C_FW1T + D] = f32("freq_w1")[0:D]
    wp[0:D, C_W2F : C_W2F + F] = f32("freq_w2")
    wp[0:D, C_W2C : C_W2C + F] = f32("comb_w2")
    wp[0:D, C_CENT : C_CENT + F] = f32("cluster_centers").T
    wp[0, C_CB1 : C_CB1 + D] = f32("comb_b1").reshape(D)
    wp[0, C_FB1 : C_FB1 + D] = f32("freq_b1").reshape(D)
    wp[0, C_B2 : C_B2 + F] = f32("comb_b2").reshape(F)
    wp[0, C_FB2 : C_FB2 + F] = f32("freq_b2").reshape(F)
    wp[D, C_FW1L : C_FW1L + D] = f32("freq_w1")[D]
    wp[D, C_TS] = float(np.asarray(inputs["total_samples"]))
    return wp


def kernel(**inputs):
    global _NC, _last_in_maps
    from concourse.bass_utils import run_bass_kernel_spmd

    ids_full = np.ascontiguousarray(
        np.asarray(inputs["category_ids"]).astype(np.int32)
    )  # [256, 64]
    common = {
        "emb_table": np.ascontiguousarray(np.asarray(inputs["emb_table"], dtype=np.float32)),
        "cat_freq": np.ascontiguousarray(
            np.asarray(inputs["cat_freq"], dtype=np.float32).reshape(V, 1)
        ),
        "wpack": _pack_weights(inputs),
    }
    in_maps = []
    for k in range(NCORES):
        m = dict(common)
        m["ids"] = np.ascontiguousarray(
            ids_full[k * B_LOC : (k + 1) * B_LOC].reshape(-1)
        )
        in_maps.append(m)

    if _NC is None:
        _NC = _build()
    _last_in_maps = in_maps
    res = run_bass_kernel_spmd(_NC, in_maps, list(range(NCORES)))
    outs = [res.results[k]["out"].reshape(B_LOC, S, F) for k in range(NCORES)]
    return np.concatenate(outs, axis=0).astype(np.float32)

